# revision 2
# baseline (speedup 1.0000x reference)
"""Dense bilinear spatial-transformer warp (nn_Dense3DSpatialTransformer) on 8 TRN2 cores.

v2: fp16 dense 7-tap formulation.  Per pixel (h, w),
    out[h,w] = sum_{dy,dx in [-3,3]} img[h+dy, w+dx] * a(dy) * b(dx)
with a(dy) = relu(1 - |fh - dy|), b(dx) = relu(1 - |fw - dx|).  This is
exact (up to fp16 rounding ~1.6e-3 rel) for every pixel with |flow| < 3;
the ~0.5% of pixels with larger |flow| are recomputed exactly on the host
from the original fp32 inputs during unsharding (they are outside the
device kernel's halo coverage).

Why fp16: DVE TensorTensor supports the 2x_1p perf mode only for 2-byte
dtypes (2 elem/lane/cycle); fp16's 11-bit mantissa keeps the weight and
product rounding ~5e-4.  All stacked tensor ops, the band, flow tiles and
the output run fp16; ACT computes weights in fp32 internally.

Layout: 128 partitions = column blocks of 32; the whole per-core image
band ([518 rows x 38 cols] fp16 per partition = 38.4 KiB) is DMA'd once
and stays resident in SBUF, so both tap shifts are static free-dim AP
offsets and row halos cost no extra DMA.  Host pre-arranges all inputs
partition-major so every DMA is a contiguous per-partition stream.

Per 64-row chunk:
  ACT:  b_dx = relu(1 - |fw - dx|)  (2 acts per dx),  a_dy planes likewise
  DVE:  ACC[j,:] = sum_dx b_dx (*) band[rows+j, cols+dx]   (7 muls + 6 adds,
        each a stacked [128,7,64,32] fp16 op in 2x mode)
  DVE:  M = A_stack (*) ACC (1 stacked mul), then a 4-op pairwise tree-add
        over the 7 dy planes (cheaper than the 1x-mode strided reduce).
"""

import time
from contextlib import ExitStack

import numpy as np

import bass_rust
import concourse.bacc as bacc
import concourse.mybir as mybir
import concourse.tile as tile

F16 = mybir.dt.float16
F32 = mybir.dt.float32

H = 4096
W = 4096
NCORES = 8
SH = H // NCORES          # 512 rows per core
HALO = 3
NTAP = 2 * HALO + 1       # 7
NPART = 128
CPB = W // NPART          # 32 columns per partition block
CPB_H = CPB + 2 * HALO    # 38 columns incl. halo
R_CHUNK = 64
BANDR = SH + 2 * HALO     # 518 rows incl. halo


def _band_stack_view(band_tile, chunk, r, dx):
    """[128, NTAP(dy), r, CPB] view of the resident band: plane j reads rows
    shifted by j, cols shifted by dx (overlapping strided AP)."""
    base = band_tile[:]
    off = base.offset + chunk * r * CPB_H + (dx + HALO)
    return bass_rust.AP(
        tensor=base.tensor, offset=off,
        ap=[list(base.ap[0]), [CPB_H, NTAP], [CPB_H, r], [1, CPB]],
    )


def _bcast_planes(ap2d, nplanes):
    """Broadcast a [128, r, c] AP across nplanes via a stride-0 plane dim."""
    return bass_rust.AP(
        tensor=ap2d.tensor, offset=ap2d.offset,
        ap=[list(ap2d.ap[0]), [0, nplanes]] + [list(d) for d in ap2d.ap[1:]],
    )


def _dram_slice(t, offset, n):
    """Contiguous [128, n] slice of a [128, N] dram tensor at elem offset."""
    ap = t.ap()
    return bass_rust.AP(tensor=ap.tensor, offset=offset,
                        ap=[list(ap.ap[0]), [1, n]])


def build_nc(sh=SH, r_chunk=R_CHUNK, debug=False):
    nc = bacc.Bacc("TRN2", target_bir_lowering=False, debug=debug)
    bandr = sh + 2 * HALO
    r = r_chunk
    n_chunks = sh // r
    assert n_chunks * r == sh

    # const APs for ACT bias values (activation looks up (F32, val))
    for v in range(-HALO - 1, HALO + 2):
        val = float(v)
        if (F32, val) not in nc.const_aps.aps:
            t = nc.alloc_sbuf_tensor(f"const-float32-{val}", [128, 1], F32)
            nc.gpsimd.memset(t.ap(), val)
            nc.const_aps.aps[(F32, val)] = t.ap()
    nc.all_engine_barrier()

    band_d = nc.dram_tensor("band", [NPART, bandr * CPB_H], F16, kind="ExternalInput")
    fh_d = nc.dram_tensor("fh", [NPART, sh * CPB], F16, kind="ExternalInput")
    fw_d = nc.dram_tensor("fw", [NPART, sh * CPB], F16, kind="ExternalInput")
    out_d = nc.dram_tensor("out", [NPART, sh * CPB], F16, kind="ExternalOutput")

    ABS = mybir.ActivationFunctionType.Abs
    RELU = mybir.ActivationFunctionType.Relu

    with tile.TileContext(nc) as tc, ExitStack() as ctx:
        band_pool = ctx.enter_context(tc.tile_pool(name="band", bufs=1))
        io_pool = ctx.enter_context(tc.tile_pool(name="io", bufs=2))
        b_pool = ctx.enter_context(tc.tile_pool(name="bwt", bufs=2))
        s_pool = ctx.enter_context(tc.tile_pool(name="stk", bufs=1))

        band = band_pool.tile([NPART, bandr, CPB_H], F16, tag="band")
        band_flat = band[:].rearrange("p r c -> p (r c)")
        for k in range(n_chunks):
            r0 = k * r
            nrow = min(r + 2 * HALO, bandr - r0)
            dst = bass_rust.AP(
                tensor=band_flat.tensor, offset=band_flat.offset + r0 * CPB_H,
                ap=[list(band_flat.ap[0]), [1, nrow * CPB_H]],
            )
            nc.sync.dma_start(dst, _dram_slice(band_d, r0 * CPB_H, nrow * CPB_H))

        for k in range(n_chunks):
            fh_t = io_pool.tile([NPART, r, CPB], F16, tag="fh")
            nc.sync.dma_start(fh_t[:].rearrange("p r c -> p (r c)"),
                              _dram_slice(fh_d, k * r * CPB, r * CPB))
            fw_t = io_pool.tile([NPART, r, CPB], F16, tag="fw")
            nc.sync.dma_start(fw_t[:].rearrange("p r c -> p (r c)"),
                              _dram_slice(fw_d, k * r * CPB, r * CPB))

            acc = s_pool.tile([NPART, NTAP, r, CPB], F16, tag="acc")
            tmp = s_pool.tile([NPART, NTAP, r, CPB], F16, tag="tmp")
            astk = s_pool.tile([NPART, NTAP, r, CPB], F16, tag="astk")

            # a_dy planes on ACT, independent of the dx loop (overlaps DVE)
            for j in range(NTAP):
                dy = j - HALO
                nc.scalar.activation(astk[:, j], fh_t[:], ABS,
                                     bias=float(-dy), scale=1.0)
                nc.scalar.activation(astk[:, j], astk[:, j], RELU,
                                     bias=1.0, scale=-1.0)

            for dxi in range(NTAP):
                dx = dxi - HALO
                b_t = b_pool.tile([NPART, r, CPB], F16, tag="b")
                nc.scalar.activation(b_t[:], fw_t[:], ABS,
                                     bias=float(-dx), scale=1.0)
                nc.scalar.activation(b_t[:], b_t[:], RELU, bias=1.0, scale=-1.0)

                bview = _bcast_planes(b_t[:], NTAP)
                sv = _band_stack_view(band, k, r, dx)
                if dxi == 0:
                    nc.vector.tensor_mul(acc[:], bview, sv)
                else:
                    nc.vector.tensor_mul(tmp[:], bview, sv)
                    nc.vector.tensor_add(acc[:], acc[:], tmp[:])

            # M = astk * acc (in place), then pairwise tree over dy planes
            nc.vector.tensor_mul(astk[:], astk[:], acc[:])
            nc.vector.tensor_add(acc[:, 0:3], astk[:, 0:3], astk[:, 4:7])
            out_t = io_pool.tile([NPART, r, CPB], F16, tag="out")
            nc.vector.tensor_add(acc[:, 4], acc[:, 0], acc[:, 1])
            nc.vector.tensor_add(acc[:, 5], acc[:, 2], astk[:, 3])
            nc.vector.tensor_add(out_t[:], acc[:, 4], acc[:, 5])

            nc.sync.dma_start(_dram_slice(out_d, k * r * CPB, r * CPB),
                              out_t[:].rearrange("p r c -> p (r c)"))

    nc.compile()
    return nc


def shard_inputs(input1, input2, sh=SH):
    img = np.asarray(input1, dtype=np.float32).reshape(H, W)
    flow = np.asarray(input2, dtype=np.float32).reshape(2, H, W)
    ncores = H // sh
    bandr = sh + 2 * HALO

    pad = np.zeros((H + 2 * HALO, W + 2 * HALO), dtype=np.float16)
    pad[HALO:H + HALO, HALO:W + HALO] = img.astype(np.float16)
    f16 = flow.astype(np.float16)

    in_maps = []
    for k in range(ncores):
        h0 = k * sh
        rows = pad[h0:h0 + bandr]                     # [bandr, W + 2*HALO]
        s0, s1 = rows.strides
        win = np.lib.stride_tricks.as_strided(
            rows, shape=(bandr, NPART, CPB_H), strides=(s0, CPB * s1, s1))
        band = np.ascontiguousarray(win.transpose(1, 0, 2)).reshape(NPART, -1)
        fh = np.ascontiguousarray(
            f16[0, h0:h0 + sh].reshape(sh, NPART, CPB).transpose(1, 0, 2)
        ).reshape(NPART, -1)
        fw = np.ascontiguousarray(
            f16[1, h0:h0 + sh].reshape(sh, NPART, CPB).transpose(1, 0, 2)
        ).reshape(NPART, -1)
        in_maps.append({"band": band, "fh": fh, "fw": fw})
    return in_maps


def _fixup_host(out, img, fh, fw):
    """Exact fp32 reference bilinear for pixels outside halo coverage."""
    f32 = np.float32
    m = (np.abs(fh) >= f32(HALO - 0.002)) | (np.abs(fw) >= f32(HALO - 0.002))
    ys, xs = np.nonzero(m)
    if len(ys) == 0:
        return
    H_up = ((fh[ys, xs] + ys.astype(f32)) + f32(1.0)).astype(f32)
    W_up = ((fw[ys, xs] + xs.astype(f32)) + f32(1.0)).astype(f32)
    Hp, Wp = H + 2, W + 2
    pad = np.zeros((Hp, Wp), f32)
    pad[1:H + 1, 1:W + 1] = img
    hf = np.floor(H_up).astype(np.int32); hc = hf + 1
    wf = np.floor(W_up).astype(np.int32); wc = wf + 1
    hf = np.clip(hf, 0, Hp - 1); hc = np.clip(hc, 0, Hp - 1)
    wf = np.clip(wf, 0, Wp - 1); wc = np.clip(wc, 0, Wp - 1)
    v00 = pad[hf, wf]; v10 = pad[hc, wf]; v01 = pad[hf, wc]; v11 = pad[hc, wc]
    dH = hc.astype(f32) - H_up
    dW = wc.astype(f32) - W_up
    out[ys, xs] = (v00 * (dW * dH) + v10 * (dW * (1 - dH))
                   + v01 * ((1 - dW) * dH) + v11 * ((1 - dW) * (1 - dH)))


_NC_CACHE = {}


def kernel(input1, input2):
    from concourse.bass_utils import run_bass_kernel_spmd

    key = (SH, R_CHUNK)
    if key not in _NC_CACHE:
        _NC_CACHE[key] = build_nc(sh=SH, r_chunk=R_CHUNK)
    nc = _NC_CACHE[key]
    in_maps = shard_inputs(input1, input2)

    last_err = None
    for attempt in range(3):
        try:
            res = run_bass_kernel_spmd(nc, in_maps, core_ids=list(range(NCORES)))
            break
        except Exception as e:  # transient device desync — retry
            last_err = e
            time.sleep(5.0 * (attempt + 1))
    else:
        raise last_err

    parts = []
    for r in res.results:
        o = r["out"].reshape(NPART, SH, CPB).transpose(1, 0, 2).reshape(SH, W)
        parts.append(o)
    out = np.concatenate(parts, axis=0).astype(np.float32)

    img = np.asarray(input1, dtype=np.float32).reshape(H, W)
    flow = np.asarray(input2, dtype=np.float32).reshape(2, H, W)
    _fixup_host(out, img, flow[0], flow[1])
    return out.reshape(1, 1, H, W).astype(np.float32)


# revision 6
# speedup vs baseline: 1.7226x; 1.7226x over previous
"""Dense bilinear spatial-transformer warp (nn_Dense3DSpatialTransformer) on 8 TRN2 cores.

v2: fp16 dense 7-tap formulation.  Per pixel (h, w),
    out[h,w] = sum_{dy,dx in [-3,3]} img[h+dy, w+dx] * a(dy) * b(dx)
with a(dy) = relu(1 - |fh - dy|), b(dx) = relu(1 - |fw - dx|).  This is
exact (up to fp16 rounding ~1.6e-3 rel) for every pixel with |flow| < 3;
the ~0.5% of pixels with larger |flow| are recomputed exactly on the host
from the original fp32 inputs during unsharding (they are outside the
device kernel's halo coverage).

Why fp16: DVE TensorTensor supports the 2x_1p perf mode only for 2-byte
dtypes (2 elem/lane/cycle); fp16's 11-bit mantissa keeps the weight and
product rounding ~5e-4.  All stacked tensor ops, the band, flow tiles and
the output run fp16; ACT computes weights in fp32 internally.

Layout: 128 partitions = column blocks of 32; the whole per-core image
band ([518 rows x 38 cols] fp16 per partition = 38.4 KiB) is DMA'd once
and stays resident in SBUF, so both tap shifts are static free-dim AP
offsets and row halos cost no extra DMA.  Host pre-arranges all inputs
partition-major so every DMA is a contiguous per-partition stream.

Per 64-row chunk:
  ACT:  b_dx = relu(1 - |fw - dx|)  (2 acts per dx),  a_dy planes likewise
  DVE:  ACC[j,:] = sum_dx b_dx (*) band[rows+j, cols+dx]   (7 muls + 6 adds,
        each a stacked [128,7,64,32] fp16 op in 2x mode)
  DVE:  M = A_stack (*) ACC (1 stacked mul), then a 4-op pairwise tree-add
        over the 7 dy planes (cheaper than the 1x-mode strided reduce).
"""

import time
from contextlib import ExitStack

import numpy as np

import bass_rust
import concourse.bacc as bacc
import concourse.mybir as mybir
import concourse.tile as tile

F16 = mybir.dt.float16
F32 = mybir.dt.float32

H = 4096
W = 4096
NCORES = 8
SH = H // NCORES          # 512 rows per core
HALO = 2
NTAP = 2 * HALO + 1       # 7
NPART = 128
CPB = W // NPART          # 32 columns per partition block
CPB_H = CPB + 2 * HALO    # 38 columns incl. halo
R_CHUNK = 64
BANDR = SH + 2 * HALO     # 518 rows incl. halo


def _band_stack_view(band_tile, chunk, r, dx):
    """[128, NTAP(dy), r, CPB] view of the resident band: plane j reads rows
    shifted by j, cols shifted by dx (overlapping strided AP)."""
    base = band_tile[:]
    off = base.offset + chunk * r * CPB_H + (dx + HALO)
    return bass_rust.AP(
        tensor=base.tensor, offset=off,
        ap=[list(base.ap[0]), [CPB_H, NTAP], [CPB_H, r], [1, CPB]],
    )


def _bcast_planes(ap2d, nplanes):
    """Broadcast a [128, r, c] AP across nplanes via a stride-0 plane dim."""
    return bass_rust.AP(
        tensor=ap2d.tensor, offset=ap2d.offset,
        ap=[list(ap2d.ap[0]), [0, nplanes]] + [list(d) for d in ap2d.ap[1:]],
    )


def _dram_slice(t, offset, n):
    """Contiguous [128, n] slice of a [128, N] dram tensor at elem offset."""
    ap = t.ap()
    return bass_rust.AP(tensor=ap.tensor, offset=offset,
                        ap=[list(ap.ap[0]), [1, n]])


def build_nc(sh=SH, r_chunk=R_CHUNK, debug=False, replay=1):
    nc = bacc.Bacc("TRN2", target_bir_lowering=False, debug=debug)
    bandr = sh + 2 * HALO
    r = r_chunk
    n_chunks = sh // r
    assert n_chunks * r == sh

    # const APs for ACT bias values (activation looks up (F32, val))
    for v in range(-HALO - 1, HALO + 2):
        val = float(v)
        if (F32, val) not in nc.const_aps.aps:
            t = nc.alloc_sbuf_tensor(f"const-float32-{val}", [128, 1], F32)
            nc.gpsimd.memset(t.ap(), val)
            nc.const_aps.aps[(F32, val)] = t.ap()
    nc.all_engine_barrier()

    band_d = nc.dram_tensor("band", [NPART, bandr * CPB_H], F16, kind="ExternalInput")
    fh_d = nc.dram_tensor("fh", [NPART, sh * CPB], F16, kind="ExternalInput")
    fw_d = nc.dram_tensor("fw", [NPART, sh * CPB], F16, kind="ExternalInput")
    out_d = nc.dram_tensor("out", [NPART, sh * CPB], F16, kind="ExternalOutput")

    ABS = mybir.ActivationFunctionType.Abs
    RELU = mybir.ActivationFunctionType.Relu

    with tile.TileContext(nc) as tc, ExitStack() as ctx:
        band_pool = ctx.enter_context(tc.tile_pool(name="band", bufs=1))
        io_pool = ctx.enter_context(tc.tile_pool(name="io", bufs=2))
        b_pool = ctx.enter_context(tc.tile_pool(name="bwt", bufs=2))
        s_pool = ctx.enter_context(tc.tile_pool(name="stk", bufs=1))

        for rp in range(replay):
          band = band_pool.tile([NPART, bandr, CPB_H], F16, tag="band")
          band_flat = band[:].rearrange("p r c -> p (r c)")
          for k in range(n_chunks):
            r0 = k * r
            nrow = min(r + 2 * HALO, bandr - r0)
            dst = bass_rust.AP(
                tensor=band_flat.tensor, offset=band_flat.offset + r0 * CPB_H,
                ap=[list(band_flat.ap[0]), [1, nrow * CPB_H]],
            )
            nc.sync.dma_start(dst, _dram_slice(band_d, r0 * CPB_H, nrow * CPB_H))

          for k in range(n_chunks):
            fh_t = io_pool.tile([NPART, r, CPB], F16, tag="fh")
            nc.sync.dma_start(fh_t[:].rearrange("p r c -> p (r c)"),
                              _dram_slice(fh_d, k * r * CPB, r * CPB))
            fw_t = io_pool.tile([NPART, r, CPB], F16, tag="fw")
            nc.sync.dma_start(fw_t[:].rearrange("p r c -> p (r c)"),
                              _dram_slice(fw_d, k * r * CPB, r * CPB))

            acc = s_pool.tile([NPART, NTAP, r, CPB], F16, tag="acc")
            tmp = s_pool.tile([NPART, NTAP, r, CPB], F16, tag="tmp")
            astk = s_pool.tile([NPART, NTAP, r, CPB], F16, tag="astk")

            # a_dy planes on ACT, independent of the dx loop (overlaps DVE)
            for j in range(NTAP):
                dy = j - HALO
                nc.scalar.activation(astk[:, j], fh_t[:], ABS,
                                     bias=float(-dy), scale=1.0)
                nc.scalar.activation(astk[:, j], astk[:, j], RELU,
                                     bias=1.0, scale=-1.0)

            for dxi in range(NTAP):
                dx = dxi - HALO
                b_t = b_pool.tile([NPART, r, CPB], F16, tag="b")
                nc.scalar.activation(b_t[:], fw_t[:], ABS,
                                     bias=float(-dx), scale=1.0)
                nc.scalar.activation(b_t[:], b_t[:], RELU, bias=1.0, scale=-1.0)

                bview = _bcast_planes(b_t[:], NTAP)
                sv = _band_stack_view(band, k, r, dx)
                if dxi == 0:
                    nc.vector.tensor_mul(acc[:], bview, sv)
                else:
                    nc.vector.tensor_mul(tmp[:], bview, sv)
                    nc.vector.tensor_add(acc[:], acc[:], tmp[:])

            # M = astk * acc (in place), then pairwise tree over dy planes
            nc.vector.tensor_mul(astk[:], astk[:], acc[:])
            out_t = io_pool.tile([NPART, r, CPB], F16, tag="out")
            if NTAP == 7:
                nc.vector.tensor_add(acc[:, 0:3], astk[:, 0:3], astk[:, 4:7])
                nc.vector.tensor_add(acc[:, 4], acc[:, 0], acc[:, 1])
                nc.vector.tensor_add(acc[:, 5], acc[:, 2], astk[:, 3])
                nc.vector.tensor_add(out_t[:], acc[:, 4], acc[:, 5])
            elif NTAP == 5:
                nc.vector.tensor_add(acc[:, 0:2], astk[:, 0:2], astk[:, 2:4])
                nc.vector.tensor_add(acc[:, 3], acc[:, 0], acc[:, 1])
                nc.vector.tensor_add(out_t[:], acc[:, 3], astk[:, 4])
            else:
                raise NotImplementedError(NTAP)

            nc.sync.dma_start(_dram_slice(out_d, k * r * CPB, r * CPB),
                              out_t[:].rearrange("p r c -> p (r c)"))

    nc.compile()
    return nc


def shard_inputs(input1, input2, sh=SH):
    img = np.asarray(input1, dtype=np.float32).reshape(H, W)
    flow = np.asarray(input2, dtype=np.float32).reshape(2, H, W)
    ncores = H // sh
    bandr = sh + 2 * HALO

    pad = np.zeros((H + 2 * HALO, W + 2 * HALO), dtype=np.float16)
    pad[HALO:H + HALO, HALO:W + HALO] = img.astype(np.float16)
    f16 = flow.astype(np.float16)

    in_maps = []
    for k in range(ncores):
        h0 = k * sh
        rows = pad[h0:h0 + bandr]                     # [bandr, W + 2*HALO]
        s0, s1 = rows.strides
        win = np.lib.stride_tricks.as_strided(
            rows, shape=(bandr, NPART, CPB_H), strides=(s0, CPB * s1, s1))
        band = np.ascontiguousarray(win.transpose(1, 0, 2)).reshape(NPART, -1)
        fh = np.ascontiguousarray(
            f16[0, h0:h0 + sh].reshape(sh, NPART, CPB).transpose(1, 0, 2)
        ).reshape(NPART, -1)
        fw = np.ascontiguousarray(
            f16[1, h0:h0 + sh].reshape(sh, NPART, CPB).transpose(1, 0, 2)
        ).reshape(NPART, -1)
        in_maps.append({"band": band, "fh": fh, "fw": fw})
    return in_maps


def _fixup_host(out, img, fh, fw):
    """Exact fp32 reference bilinear for pixels outside halo coverage."""
    f32 = np.float32
    m = (np.abs(fh) >= f32(HALO - 0.002)) | (np.abs(fw) >= f32(HALO - 0.002))
    ys, xs = np.nonzero(m)
    if len(ys) == 0:
        return
    H_up = ((fh[ys, xs] + ys.astype(f32)) + f32(1.0)).astype(f32)
    W_up = ((fw[ys, xs] + xs.astype(f32)) + f32(1.0)).astype(f32)
    Hp, Wp = H + 2, W + 2
    pad = np.zeros((Hp, Wp), f32)
    pad[1:H + 1, 1:W + 1] = img
    hf = np.floor(H_up).astype(np.int32); hc = hf + 1
    wf = np.floor(W_up).astype(np.int32); wc = wf + 1
    hf = np.clip(hf, 0, Hp - 1); hc = np.clip(hc, 0, Hp - 1)
    wf = np.clip(wf, 0, Wp - 1); wc = np.clip(wc, 0, Wp - 1)
    v00 = pad[hf, wf]; v10 = pad[hc, wf]; v01 = pad[hf, wc]; v11 = pad[hc, wc]
    dH = hc.astype(f32) - H_up
    dW = wc.astype(f32) - W_up
    out[ys, xs] = (v00 * (dW * dH) + v10 * (dW * (1 - dH))
                   + v01 * ((1 - dW) * dH) + v11 * ((1 - dW) * (1 - dH)))


_NC_CACHE = {}


def kernel(input1, input2):
    from concourse.bass_utils import run_bass_kernel_spmd

    key = (SH, R_CHUNK)
    if key not in _NC_CACHE:
        _NC_CACHE[key] = build_nc(sh=SH, r_chunk=R_CHUNK)
    nc = _NC_CACHE[key]
    in_maps = shard_inputs(input1, input2)

    last_err = None
    for attempt in range(3):
        try:
            res = run_bass_kernel_spmd(nc, in_maps, core_ids=list(range(NCORES)))
            break
        except Exception as e:  # transient device desync — retry
            last_err = e
            time.sleep(5.0 * (attempt + 1))
    else:
        raise last_err

    parts = []
    for r in res.results:
        o = r["out"].reshape(NPART, SH, CPB).transpose(1, 0, 2).reshape(SH, W)
        parts.append(o)
    out = np.concatenate(parts, axis=0).astype(np.float32)

    img = np.asarray(input1, dtype=np.float32).reshape(H, W)
    flow = np.asarray(input2, dtype=np.float32).reshape(2, H, W)
    _fixup_host(out, img, flow[0], flow[1])
    return out.reshape(1, 1, H, W).astype(np.float32)
